# revision 13
# baseline (speedup 1.0000x reference)
"""Prefix-causal multi-head self-attention on 8 Trainium2 NeuronCores.

Sharding: data parallel over batch (B=2) x tensor parallel over heads
(16 heads -> 4 groups of 4). Core c handles batch c//4, head group c%4.
Each core computes its heads' contribution to the output projection as a
partial [T, C] sum; the host adds the 4 partials per batch plus bproj.

All matmuls run in float32r (TF32-like, 1 PE cycle/row at N>=256).
Attention processes head pairs: head 2p in array rows 0-63, head 2p+1 in
rows 64-127 (disjoint row groups overlap on the PE), with both heads'
scores sharing one 2-bank PSUM tile so softmax exp is a single ACT op.
"""
import sys

sys.path.insert(0, "/opt/trn_rl_repo")

import numpy as np

import concourse.tile as tile
from concourse import bacc, mybir
from concourse.bass_utils import run_bass_kernel_spmd

F32 = mybir.dt.float32
F32R = mybir.dt.float32r
EXP = mybir.ActivationFunctionType.Exp
IDENT = mybir.ActivationFunctionType.Identity
MULT = mybir.AluOpType.mult
ADD = mybir.AluOpType.add

B, T, C = 2, 2048, 1024
H, D = 16, 64          # heads, head dim
NT = T // 128          # 16 key/query tiles
NQC = T // 512         # 4 query chunks
NCCH = C // 128        # 8 contraction chunks

_cache = {}


def _build():
    nc = bacc.Bacc("TRN2", target_bir_lowering=False, debug=False)

    xT = nc.dram_tensor("xT", [C, T], F32R, kind="ExternalInput").ap()
    wqk = nc.dram_tensor("wqk", [C, 512], F32R, kind="ExternalInput").ap()
    bqk = nc.dram_tensor("bqk", [4, 128, 1], F32, kind="ExternalInput").ap()
    wv = nc.dram_tensor("wv", [C, 256], F32R, kind="ExternalInput").ap()
    bv = nc.dram_tensor("bv", [1, 256], F32, kind="ExternalInput").ap()
    wproj = nc.dram_tensor("wproj", [2, 128, C], F32R, kind="ExternalInput").ap()
    tri2 = nc.dram_tensor("tri2", [128, 256], F32R, kind="ExternalInput").ap()
    out = nc.dram_tensor("out", [T, C], F32, kind="ExternalOutput").ap()

    with tile.TileContext(nc) as tc:
        with tc.tile_pool(name="persist", bufs=1) as pp, \
             tc.tile_pool(name="attn", bufs=3) as apool, \
             tc.tile_pool(name="rbcp", bufs=2) as rbp, \
             tc.tile_pool(name="r0p", bufs=2) as r0p, \
             tc.tile_pool(name="psS", bufs=2, space="PSUM") as psS, \
             tc.tile_pool(name="psY", bufs=1, space="PSUM") as psY, \
             tc.tile_pool(name="psP", bufs=1, space="PSUM") as psP:
            # ---- persistent tiles ----
            xT_sb = [pp.tile([128, T], F32R, tag=f"xt{c}", name=f"xt{c}")
                     for c in range(NCCH)]
            wqkwv_pool = tc.tile_pool(name="wqkwv", bufs=1)
            wpp = wqkwv_pool.__enter__()
            wqk_sb = [wpp.tile([128, 512], F32R, tag=f"wqk{c}", name=f"wqk{c}")
                      for c in range(NCCH)]
            wv_sb = [wpp.tile([128, 256], F32R, tag=f"wv{c}", name=f"wv{c}")
                     for c in range(NCCH)]
            wproj_sb = [pp.tile([128, C], F32R, tag=f"wp{p}", name=f"wp{p}")
                        for p in range(2)]
            bqk_sb = [pp.tile([128, 1], F32, tag=f"bqk{m}", name=f"bqk{m}")
                      for m in range(4)]
            bv_sb = pp.tile([128, 256], F32, tag="bv")
            tri_sb = pp.tile([128, 256], F32R, tag="tri")
            ones4 = pp.tile([128, 4, 1], F32, tag="ones4")
            qk_sb = [pp.tile([128, T], F32R, tag=f"qk{m}", name=f"qk{m}")
                     for m in range(4)]
            v_sb = [pp.tile([128, 4 * 65], F32R, tag=f"v{kt}", name=f"v{kt}")
                    for kt in range(NT)]
            y_sb = [pp.tile([128, T], F32R, tag=f"y{p}", name=f"y{p}")
                    for p in range(2)]

            # ---- input DMAs (xT/wqk pairs first so phase B starts early) ----
            for c in range(NCCH):
                nc.sync.dma_start(xT_sb[c][:], xT[c * 128:(c + 1) * 128, :])
                nc.sync.dma_start(wqk_sb[c][:], wqk[c * 128:(c + 1) * 128, :])
            for c in range(NCCH):
                nc.sync.dma_start(wv_sb[c][:], wv[c * 128:(c + 1) * 128, :])
            for m in range(4):
                nc.sync.dma_start(bqk_sb[m][:], bqk[m])
            nc.gpsimd.dma_start(bv_sb[:], bv.partition_broadcast(128))
            nc.sync.dma_start(tri_sb[:], tri2[:])
            for p in range(2):
                nc.sync.dma_start(wproj_sb[p][:], wproj[p])
            nc.vector.memset(ones4[:], 1.0)

            # ---- PE warm-up: dummy bf16 matmuls during the initial DMA so
            # the HAM clock-gate releases (1.2 -> 2.4 GHz) before phase B ----
            warm_sb = pp.tile([128, 512], mybir.dt.bfloat16, tag="warm")
            nc.vector.memset(warm_sb[:], 0.0)
            for i in range(30):
                pw = psP.tile([128, 1024], F32, tag="po", name="pw")
                nc.tensor.matmul(pw[:, 0:512], warm_sb[:, 0:128], warm_sb[:],
                                 start=True, stop=True)

            # ---- phase B: qkT = wqk.T @ xT + bqk  -> [512 rows, T] ----
            for m in range(4):
                for half in range(2):
                    pb = psS.tile([128, 1024], F32, tag="ps", name="pb")
                    for c in range(NCCH):
                        for nn in range(2):
                            nc.tensor.matmul(
                                pb[:, nn * 512:(nn + 1) * 512],
                                wqk_sb[c][:, m * 128:(m + 1) * 128],
                                xT_sb[c][:, (2 * half + nn) * 512:
                                          (2 * half + nn + 1) * 512],
                                start=(c == 0), stop=(c == NCCH - 1),
                            )
                    nc.scalar.activation(
                        qk_sb[m][:, half * 1024:(half + 1) * 1024], pb[:],
                        IDENT, bias=bqk_sb[m][:], scale=1.0,
                    )

            # ---- phase C: v = xT.T @ wv + bv -> [T, 256] (+ ones cols) ----
            for kt in range(NT):
                pv = psS.tile([128, 256], F32, tag="ps", name="pv")
                for c in range(NCCH):
                    nc.tensor.matmul(
                        pv[:],
                        xT_sb[c][:, kt * 128:(kt + 1) * 128],
                        wv_sb[c][:],
                        start=(c == 0), stop=(c == NCCH - 1),
                    )
                vt = v_sb[kt][:].rearrange("p (h e) -> p h e", e=65)
                nc.vector.tensor_tensor(
                    vt[:, :, 0:64],
                    pv[:].rearrange("p (h e) -> p h e", e=64),
                    bv_sb[:].rearrange("p (h e) -> p h e", e=64),
                    op=ADD,
                )
                nc.vector.tensor_copy(vt[:, :, 64:65], ones4[:])

            wqkwv_pool.__exit__(None, None, None)
            yun_pool = tc.tile_pool(name="yun", bufs=2)
            yup = yun_pool.__enter__()
            osp_pool = tc.tile_pool(name="ostage", bufs=2)
            osp = osp_pool.__enter__()

            # ---- attention (head pair p: head 2p rows 0:64, 2p+1 rows 64:128) ----
            def emit_scores(p, qc, kj):
                q_t, k_t = qk_sb[2 * p], qk_sb[2 * p + 1]
                off = max(0, kj * 128 - qc * 512)
                nv = 512 - off
                ps = psS.tile([128, 1024], F32, tag="ps", name="ps")
                for hl in range(2):
                    r = slice(64 * hl, 64 * hl + 64)
                    nc.tensor.matmul(
                        ps[:, hl * 512:hl * 512 + nv],
                        k_t[r, kj * 128:(kj + 1) * 128],
                        q_t[r, qc * 512 + off:(qc + 1) * 512],
                        start=True, stop=True,
                    )
                at = apool.tile([128, 1024], F32R, tag="at", name="at")
                nc.scalar.activation(
                    at[:].rearrange("p (h n) -> p h n", h=2)[:, :, 0:nv],
                    ps[:].rearrange("p (h n) -> p h n", h=2)[:, :, 0:nv],
                    EXP, bias=0.0, scale=0.125,
                )
                if kj * 128 >= qc * 512 and kj != 0:
                    # diagonal block: causal triangle (both heads in one op;
                    # kj == 0 is the fully-visible S prefix)
                    nc.vector.tensor_tensor(
                        at[:].rearrange("p (h n) -> p h n", h=2)[:, :, 0:128],
                        at[:].rearrange("p (h n) -> p h n", h=2)[:, :, 0:128],
                        tri_sb[:].rearrange("p (h n) -> p h n", h=2),
                        op=MULT,
                    )
                return at, off, nv

            def emit_av(p, qc, kj, py, at, off, nv, nkj):
                for hl in range(2):
                    hc = 2 * p + hl
                    nc.tensor.matmul(
                        py[:, hl * 512 + off:(hl + 1) * 512],
                        v_sb[kj][:, hc * 65:(hc + 1) * 65],
                        at[:, hl * 512:hl * 512 + nv],
                        start=(kj == 0), stop=(kj == nkj - 1),
                    )

            def emit_proj(qi):
                po = psP.tile([128, 1024], F32, tag="po", name="po")
                for p in range(2):
                    for ch in range(2):
                        nc.tensor.matmul(
                            po[:, ch * 512:(ch + 1) * 512],
                            y_sb[p][:, qi * 128:(qi + 1) * 128],
                            wproj_sb[p][:, ch * 512:(ch + 1) * 512],
                            start=(p == 0), stop=(p == 1),
                        )
                o_sb = osp.tile([128, C], F32, tag="o_sb", name="o_sb")
                nc.vector.tensor_copy(o_sb[:], po[:])
                nc.sync.dma_start(out[qi * 128:(qi + 1) * 128, :], o_sb[:])

            for p in range(2):
                for qc in range(NQC):
                    py = psY.tile([65, 1024], F32, tag="py", name="py")
                    nkj = 4 * qc + 4
                    # software pipeline: scores(kj+1) emitted before AV(kj)
                    pend = None
                    for kj in range(nkj):
                        cur = (kj, *emit_scores(p, qc, kj))
                        if pend is not None:
                            pkj, pat, poff, pnv = pend
                            emit_av(p, qc, pkj, py, pat, poff, pnv, nkj)
                        pend = cur
                    pkj, pat, poff, pnv = pend
                    emit_av(p, qc, pkj, py, pat, poff, pnv, nkj)
                    # normalize: y = y_un / denom (denom = row 64).
                    # Copy PSUM -> SBUF first so the psum slot frees early;
                    # the divide chain then runs off the PE critical path.
                    yun = yup.tile([65, 1024], F32, tag="yun", name="yun")
                    nc.vector.tensor_copy(yun[:], py[:])
                    r0 = r0p.tile([1, 1024], F32, tag="r0")
                    nc.vector.reciprocal_approx_fast(r0[:], yun[64:65, :])
                    rb = rbp.tile([64, 1024], F32, tag="rb")
                    nc.gpsimd.partition_broadcast(rb[:], r0[:])
                    for hl in range(2):
                        nc.vector.tensor_tensor(
                            y_sb[p][64 * hl:64 * hl + 64,
                                    qc * 512:(qc + 1) * 512],
                            yun[0:64, hl * 512:(hl + 1) * 512],
                            rb[:, hl * 512:(hl + 1) * 512], op=MULT,
                        )
                    if p == 1:
                        # pair-1 y chunk done: its projection rows are ready
                        # (pair-0 y fully done) -> overlap proj + output DMA
                        # with the rest of pair-1 attention
                        for qi in range(4 * qc, 4 * qc + 4):
                            emit_proj(qi)

            osp_pool.__exit__(None, None, None)
            yun_pool.__exit__(None, None, None)

    nc.compile()
    return nc


def _prep_inputs(x, Wqkv, bqkv, Wproj):
    """Per-core input maps. Core c: batch c//4, head group c%4."""
    xT = [np.ascontiguousarray(x[b].T) for b in range(B)]
    tri = np.triu(np.ones((128, 128), np.float32))
    tri2 = np.ascontiguousarray(np.concatenate([tri, tri], axis=1))
    per_group = []
    for g in range(4):
        hs = [4 * g + i for i in range(4)]
        # wqk col order: q_h0 q_h1 k_h0 k_h1 q_h2 q_h3 k_h2 k_h3
        cols, bcols = [], []
        for pair in (hs[0:2], hs[2:4]):
            for qk_off in (0, C):
                for h in pair:
                    cols.append(Wqkv[:, qk_off + 64 * h:qk_off + 64 * h + 64])
                    bcols.append(bqkv[qk_off + 64 * h:qk_off + 64 * h + 64])
        wqk = np.ascontiguousarray(np.concatenate(cols, axis=1))
        bqk = np.concatenate(bcols).reshape(4, 128, 1).astype(np.float32)
        wv = np.ascontiguousarray(
            np.concatenate([Wqkv[:, 2 * C + 64 * h:2 * C + 64 * h + 64]
                            for h in hs], axis=1))
        bvs = np.concatenate([bqkv[2 * C + 64 * h:2 * C + 64 * h + 64]
                              for h in hs]).reshape(1, 256).astype(np.float32)
        wp = np.stack([
            np.concatenate([Wproj[64 * h:64 * h + 64] for h in hs[0:2]], axis=0),
            np.concatenate([Wproj[64 * h:64 * h + 64] for h in hs[2:4]], axis=0),
        ]).astype(np.float32)
        per_group.append((wqk, bqk, wv, bvs, wp))
    in_maps = []
    for c in range(8):
        b, g = c // 4, c % 4
        wqk, bqk, wv, bvs, wp = per_group[g]
        in_maps.append({
            "xT": xT[b], "wqk": wqk, "bqk": bqk, "wv": wv, "bv": bvs,
            "wproj": wp, "tri2": tri2,
        })
    return in_maps


def run_traced(inputs):
    """Harness helper: one traced run returning BassKernelResults."""
    if "nc" not in _cache:
        _cache["nc"] = _build()
    in_maps = _prep_inputs(
        np.asarray(inputs["x"], np.float32),
        np.asarray(inputs["Wqkv"], np.float32),
        np.asarray(inputs["bqkv"], np.float32),
        np.asarray(inputs["Wproj"], np.float32))
    return run_bass_kernel_spmd(_cache["nc"], in_maps,
                                core_ids=list(range(8)), trace=True)


def kernel(x, Wqkv, bqkv, Wproj, bproj, S):
    assert int(S) == 128, f"kernel specialized for S=128, got {S}"
    x = np.asarray(x, np.float32)
    Wqkv = np.asarray(Wqkv, np.float32)
    bqkv = np.asarray(bqkv, np.float32)
    Wproj = np.asarray(Wproj, np.float32)
    bproj = np.asarray(bproj, np.float32)

    if "nc" not in _cache:
        _cache["nc"] = _build()
    nc = _cache["nc"]

    in_maps = _prep_inputs(x, Wqkv, bqkv, Wproj)
    res = run_bass_kernel_spmd(nc, in_maps, core_ids=list(range(8)))
    out = np.empty((B, T, C), np.float32)
    for b in range(B):
        acc = res.results[4 * b]["out"].astype(np.float64)
        for g in range(1, 4):
            acc += res.results[4 * b + g]["out"]
        out[b] = (acc + bproj).astype(np.float32)
    return out


# revision 14
# speedup vs baseline: 1.0864x; 1.0864x over previous
"""Prefix-causal multi-head self-attention on 8 Trainium2 NeuronCores.

Sharding: data parallel over batch (B=2) x tensor parallel over heads
(16 heads -> 4 groups of 4). Core c handles batch c//4, head group c%4.
Each core computes its heads' contribution to the output projection as a
partial [T, C] sum; the host adds the 4 partials per batch plus bproj.

All matmuls run in float32r (TF32-like, 1 PE cycle/row at N>=256).
Attention processes head pairs: head 2p in array rows 0-63, head 2p+1 in
rows 64-127 (disjoint row groups overlap on the PE), with both heads'
scores sharing one 2-bank PSUM tile so softmax exp is a single ACT op.
"""
import sys

sys.path.insert(0, "/opt/trn_rl_repo")

import numpy as np

import concourse.tile as tile
from concourse import bacc, mybir
from concourse.bass_utils import run_bass_kernel_spmd

F32 = mybir.dt.float32
F32R = mybir.dt.float32r
BF16 = mybir.dt.bfloat16
BF16_IN = True   # x / Wqkv / Wv inputs in bf16 (halves input DMA)
IN_DT = BF16 if BF16_IN else F32R
EXP = mybir.ActivationFunctionType.Exp
IDENT = mybir.ActivationFunctionType.Identity
MULT = mybir.AluOpType.mult
ADD = mybir.AluOpType.add

B, T, C = 2, 2048, 1024
H, D = 16, 64          # heads, head dim
NT = T // 128          # 16 key/query tiles
NQC = T // 512         # 4 query chunks
NCCH = C // 128        # 8 contraction chunks

_cache = {}


def _build():
    nc = bacc.Bacc("TRN2", target_bir_lowering=False, debug=False)

    xT = nc.dram_tensor("xT", [C, T], IN_DT, kind="ExternalInput").ap()
    wqk = nc.dram_tensor("wqk", [C, 512], IN_DT, kind="ExternalInput").ap()
    bqk = nc.dram_tensor("bqk", [4, 128, 1], F32, kind="ExternalInput").ap()
    wv = nc.dram_tensor("wv", [C, 256], IN_DT, kind="ExternalInput").ap()
    bv = nc.dram_tensor("bv", [1, 256], F32, kind="ExternalInput").ap()
    wproj = nc.dram_tensor("wproj", [2, 128, C], F32R, kind="ExternalInput").ap()
    tri2 = nc.dram_tensor("tri2", [128, 256], F32R, kind="ExternalInput").ap()
    out = nc.dram_tensor("out", [T, C], F32, kind="ExternalOutput").ap()

    with tile.TileContext(nc) as tc:
        with tc.tile_pool(name="persist", bufs=1) as pp, \
             tc.tile_pool(name="attn", bufs=3) as apool, \
             tc.tile_pool(name="rbcp", bufs=2) as rbp, \
             tc.tile_pool(name="r0p", bufs=2) as r0p, \
             tc.tile_pool(name="psS", bufs=2, space="PSUM") as psS, \
             tc.tile_pool(name="psY", bufs=1, space="PSUM") as psY, \
             tc.tile_pool(name="psP", bufs=1, space="PSUM") as psP:
            # ---- persistent tiles ----
            xT_sb = [pp.tile([128, T], IN_DT, tag=f"xt{c}", name=f"xt{c}")
                     for c in range(NCCH)]
            wqkwv_pool = tc.tile_pool(name="wqkwv", bufs=1)
            wpp = wqkwv_pool.__enter__()
            wqk_sb = [wpp.tile([128, 512], IN_DT, tag=f"wqk{c}", name=f"wqk{c}")
                      for c in range(NCCH)]
            wv_sb = [wpp.tile([128, 256], IN_DT, tag=f"wv{c}", name=f"wv{c}")
                     for c in range(NCCH)]
            wproj_sb = [pp.tile([128, C], F32R, tag=f"wp{p}", name=f"wp{p}")
                        for p in range(2)]
            bqk_sb = [pp.tile([128, 1], F32, tag=f"bqk{m}", name=f"bqk{m}")
                      for m in range(4)]
            bv_sb = pp.tile([128, 256], F32, tag="bv")
            tri_sb = pp.tile([128, 256], F32R, tag="tri")
            ones4 = pp.tile([128, 4, 1], F32, tag="ones4")
            qk_sb = [pp.tile([128, T], F32R, tag=f"qk{m}", name=f"qk{m}")
                     for m in range(4)]
            v_sb = [pp.tile([128, 4 * 65], F32R, tag=f"v{kt}", name=f"v{kt}")
                    for kt in range(NT)]
            y_sb = [pp.tile([128, T], F32R, tag=f"y{p}", name=f"y{p}")
                    for p in range(2)]

            # ---- input DMAs (xT/wqk pairs first so phase B starts early) ----
            for c in range(NCCH):
                nc.sync.dma_start(xT_sb[c][:], xT[c * 128:(c + 1) * 128, :])
                nc.sync.dma_start(wqk_sb[c][:], wqk[c * 128:(c + 1) * 128, :])
            for c in range(NCCH):
                nc.sync.dma_start(wv_sb[c][:], wv[c * 128:(c + 1) * 128, :])
            for m in range(4):
                nc.sync.dma_start(bqk_sb[m][:], bqk[m])
            nc.gpsimd.dma_start(bv_sb[:], bv.partition_broadcast(128))
            nc.sync.dma_start(tri_sb[:], tri2[:])
            for p in range(2):
                nc.sync.dma_start(wproj_sb[p][:], wproj[p])
            nc.vector.memset(ones4[:], 1.0)

            # ---- PE warm-up: dummy bf16 matmuls during the initial DMA so
            # the HAM clock-gate releases (1.2 -> 2.4 GHz) before phase B ----
            warm_sb = pp.tile([128, 512], mybir.dt.bfloat16, tag="warm")
            nc.vector.memset(warm_sb[:], 0.0)
            for i in range(30):
                pw = psP.tile([128, 1024], F32, tag="po", name="pw")
                nc.tensor.matmul(pw[:, 0:512], warm_sb[:, 0:128], warm_sb[:],
                                 start=True, stop=True)

            # ---- phase B: qkT = wqk.T @ xT + bqk  -> [512 rows, T] ----
            for m in range(4):
                for half in range(2):
                    pb = psS.tile([128, 1024], F32, tag="ps", name="pb")
                    for c in range(NCCH):
                        for nn in range(2):
                            nc.tensor.matmul(
                                pb[:, nn * 512:(nn + 1) * 512],
                                wqk_sb[c][:, m * 128:(m + 1) * 128],
                                xT_sb[c][:, (2 * half + nn) * 512:
                                          (2 * half + nn + 1) * 512],
                                start=(c == 0), stop=(c == NCCH - 1),
                            )
                    nc.scalar.activation(
                        qk_sb[m][:, half * 1024:(half + 1) * 1024], pb[:],
                        IDENT, bias=bqk_sb[m][:], scale=1.0,
                    )

            # ---- phase C: v = xT.T @ wv + bv -> [T, 256] (+ ones cols) ----
            for kt in range(NT):
                pv = psS.tile([128, 256], F32, tag="ps", name="pv")
                for c in range(NCCH):
                    nc.tensor.matmul(
                        pv[:],
                        xT_sb[c][:, kt * 128:(kt + 1) * 128],
                        wv_sb[c][:],
                        start=(c == 0), stop=(c == NCCH - 1),
                    )
                vt = v_sb[kt][:].rearrange("p (h e) -> p h e", e=65)
                nc.vector.tensor_tensor(
                    vt[:, :, 0:64],
                    pv[:].rearrange("p (h e) -> p h e", e=64),
                    bv_sb[:].rearrange("p (h e) -> p h e", e=64),
                    op=ADD,
                )
                nc.vector.tensor_copy(vt[:, :, 64:65], ones4[:])

            wqkwv_pool.__exit__(None, None, None)
            yun_pool = tc.tile_pool(name="yun", bufs=2)
            yup = yun_pool.__enter__()
            osp_pool = tc.tile_pool(name="ostage", bufs=2)
            osp = osp_pool.__enter__()

            # ---- attention (head pair p: head 2p rows 0:64, 2p+1 rows 64:128) ----
            def emit_scores(p, qc, kj):
                q_t, k_t = qk_sb[2 * p], qk_sb[2 * p + 1]
                off = max(0, kj * 128 - qc * 512)
                nv = 512 - off
                ps = psS.tile([128, 1024], F32, tag="ps", name="ps")
                for hl in range(2):
                    r = slice(64 * hl, 64 * hl + 64)
                    nc.tensor.matmul(
                        ps[:, hl * 512:hl * 512 + nv],
                        k_t[r, kj * 128:(kj + 1) * 128],
                        q_t[r, qc * 512 + off:(qc + 1) * 512],
                        start=True, stop=True,
                    )
                at = apool.tile([128, 1024], F32R, tag="at", name="at")
                nc.scalar.activation(
                    at[:].rearrange("p (h n) -> p h n", h=2)[:, :, 0:nv],
                    ps[:].rearrange("p (h n) -> p h n", h=2)[:, :, 0:nv],
                    EXP, bias=0.0, scale=0.125,
                )
                if kj * 128 >= qc * 512 and kj != 0:
                    # diagonal block: causal triangle (both heads in one op;
                    # kj == 0 is the fully-visible S prefix)
                    nc.vector.tensor_tensor(
                        at[:].rearrange("p (h n) -> p h n", h=2)[:, :, 0:128],
                        at[:].rearrange("p (h n) -> p h n", h=2)[:, :, 0:128],
                        tri_sb[:].rearrange("p (h n) -> p h n", h=2),
                        op=MULT,
                    )
                return at, off, nv

            def emit_av(p, qc, kj, py, at, off, nv, nkj):
                for hl in range(2):
                    hc = 2 * p + hl
                    nc.tensor.matmul(
                        py[:, hl * 512 + off:(hl + 1) * 512],
                        v_sb[kj][:, hc * 65:(hc + 1) * 65],
                        at[:, hl * 512:hl * 512 + nv],
                        start=(kj == 0), stop=(kj == nkj - 1),
                    )

            def emit_proj(qi):
                po = psP.tile([128, 1024], F32, tag="po", name="po")
                for p in range(2):
                    for ch in range(2):
                        nc.tensor.matmul(
                            po[:, ch * 512:(ch + 1) * 512],
                            y_sb[p][:, qi * 128:(qi + 1) * 128],
                            wproj_sb[p][:, ch * 512:(ch + 1) * 512],
                            start=(p == 0), stop=(p == 1),
                        )
                o_sb = osp.tile([128, C], F32, tag="o_sb", name="o_sb")
                nc.vector.tensor_copy(o_sb[:], po[:])
                nc.sync.dma_start(out[qi * 128:(qi + 1) * 128, :], o_sb[:])

            for p in range(2):
                for qc in range(NQC):
                    py = psY.tile([65, 1024], F32, tag="py", name="py")
                    nkj = 4 * qc + 4
                    # software pipeline: scores(kj+1) emitted before AV(kj)
                    pend = None
                    for kj in range(nkj):
                        cur = (kj, *emit_scores(p, qc, kj))
                        if pend is not None:
                            pkj, pat, poff, pnv = pend
                            emit_av(p, qc, pkj, py, pat, poff, pnv, nkj)
                        pend = cur
                    pkj, pat, poff, pnv = pend
                    emit_av(p, qc, pkj, py, pat, poff, pnv, nkj)
                    # normalize: y = y_un / denom (denom = row 64).
                    # Copy PSUM -> SBUF first so the psum slot frees early;
                    # the divide chain then runs off the PE critical path.
                    yun = yup.tile([65, 1024], F32, tag="yun", name="yun")
                    nc.vector.tensor_copy(yun[:], py[:])
                    r0 = r0p.tile([1, 1024], F32, tag="r0")
                    nc.vector.reciprocal_approx_fast(r0[:], yun[64:65, :])
                    rb = rbp.tile([64, 1024], F32, tag="rb")
                    nc.gpsimd.partition_broadcast(rb[:], r0[:])
                    for hl in range(2):
                        nc.vector.tensor_tensor(
                            y_sb[p][64 * hl:64 * hl + 64,
                                    qc * 512:(qc + 1) * 512],
                            yun[0:64, hl * 512:(hl + 1) * 512],
                            rb[:, hl * 512:(hl + 1) * 512], op=MULT,
                        )
                    if p == 1:
                        # pair-1 y chunk done: its projection rows are ready
                        # (pair-0 y fully done) -> overlap proj + output DMA
                        # with the rest of pair-1 attention
                        for qi in range(4 * qc, 4 * qc + 4):
                            emit_proj(qi)

            osp_pool.__exit__(None, None, None)
            yun_pool.__exit__(None, None, None)

    nc.compile()
    return nc


def _in_cast(a):
    if BF16_IN:
        import ml_dtypes
        return np.ascontiguousarray(a).astype(ml_dtypes.bfloat16)
    return np.ascontiguousarray(a)


def _prep_inputs(x, Wqkv, bqkv, Wproj):
    """Per-core input maps. Core c: batch c//4, head group c%4."""
    xT = [_in_cast(x[b].T) for b in range(B)]
    tri = np.triu(np.ones((128, 128), np.float32))
    tri2 = np.ascontiguousarray(np.concatenate([tri, tri], axis=1))
    per_group = []
    for g in range(4):
        hs = [4 * g + i for i in range(4)]
        # wqk col order: q_h0 q_h1 k_h0 k_h1 q_h2 q_h3 k_h2 k_h3
        cols, bcols = [], []
        for pair in (hs[0:2], hs[2:4]):
            for qk_off in (0, C):
                for h in pair:
                    cols.append(Wqkv[:, qk_off + 64 * h:qk_off + 64 * h + 64])
                    bcols.append(bqkv[qk_off + 64 * h:qk_off + 64 * h + 64])
        wqk = _in_cast(np.concatenate(cols, axis=1))
        bqk = np.concatenate(bcols).reshape(4, 128, 1).astype(np.float32)
        wv = _in_cast(
            np.concatenate([Wqkv[:, 2 * C + 64 * h:2 * C + 64 * h + 64]
                            for h in hs], axis=1))
        bvs = np.concatenate([bqkv[2 * C + 64 * h:2 * C + 64 * h + 64]
                              for h in hs]).reshape(1, 256).astype(np.float32)
        wp = np.stack([
            np.concatenate([Wproj[64 * h:64 * h + 64] for h in hs[0:2]], axis=0),
            np.concatenate([Wproj[64 * h:64 * h + 64] for h in hs[2:4]], axis=0),
        ]).astype(np.float32)
        per_group.append((wqk, bqk, wv, bvs, wp))
    in_maps = []
    for c in range(8):
        b, g = c // 4, c % 4
        wqk, bqk, wv, bvs, wp = per_group[g]
        in_maps.append({
            "xT": xT[b], "wqk": wqk, "bqk": bqk, "wv": wv, "bv": bvs,
            "wproj": wp, "tri2": tri2,
        })
    return in_maps


def run_traced(inputs):
    """Harness helper: one traced run returning BassKernelResults."""
    if "nc" not in _cache:
        _cache["nc"] = _build()
    in_maps = _prep_inputs(
        np.asarray(inputs["x"], np.float32),
        np.asarray(inputs["Wqkv"], np.float32),
        np.asarray(inputs["bqkv"], np.float32),
        np.asarray(inputs["Wproj"], np.float32))
    return run_bass_kernel_spmd(_cache["nc"], in_maps,
                                core_ids=list(range(8)), trace=True)


def kernel(x, Wqkv, bqkv, Wproj, bproj, S):
    assert int(S) == 128, f"kernel specialized for S=128, got {S}"
    x = np.asarray(x, np.float32)
    Wqkv = np.asarray(Wqkv, np.float32)
    bqkv = np.asarray(bqkv, np.float32)
    Wproj = np.asarray(Wproj, np.float32)
    bproj = np.asarray(bproj, np.float32)

    if "nc" not in _cache:
        _cache["nc"] = _build()
    nc = _cache["nc"]

    in_maps = _prep_inputs(x, Wqkv, bqkv, Wproj)
    res = run_bass_kernel_spmd(nc, in_maps, core_ids=list(range(8)))
    out = np.empty((B, T, C), np.float32)
    for b in range(B):
        acc = res.results[4 * b]["out"].astype(np.float64)
        for g in range(1, 4):
            acc += res.results[4 * b + g]["out"]
        out[b] = (acc + bproj).astype(np.float32)
    return out
